# revision 22
# baseline (speedup 1.0000x reference)
"""ArcFace head (B=1024, D=512, C=100000) on 8 TRN2 NeuronCores.

Sharding: tensor-parallel along num_classes (partial-FC ArcFace). Each core
computes a [B, 12288] slice of S*cosine; the ragged 212-col remainder and the
per-row margin are handled on the host.

Hybrid precision (key speed lever, tuned against the 2e-2 rel-err gate):
- cols 0:3072   : fp8 e4m3 with perf_mode=DoubleRow (2 k-subtiles per MM;
                  a DR matmul measures the same 216ns as a bf16 N=512 MM =
                  clean 2x throughput). Host pre-scales both operands by 16
                  (e4m3 normal range); the S/256 descale is folded into the
                  PSUM->SBUF copy (ACT mul / DVE tensor_scalar_mul).
                  Device rel-err matches the host ml_dtypes sim to 6
                  digits: 1.939e-2 (gate 2e-2, bf16-only is 2.9e-3).
- cols 3072:12288: bf16, the traced-gapless baseline structure (m-outer
                  k-inner supertiles of 1024, 216ns/MM warm roofline).
- label-column logits are recomputed exactly on host before the margin, so
  fp8 noise never feeds the acos/cos margin transform.

Edge schedule (from per-run NTFF traces: ~7.2us fixed NEFF init, each
dma_start trigger ~0.65us serial on its engine SEQUENCER, doorbell->data
~0.8us, early DMA bandwidth ramps slowly and sub-1KB-run descriptors load
far slower than 1KB+ runs):
- ALL early inputs ride the sync queue in exact consumption order (FIFO
  within one queue is the only ordering guarantee — the Tile scheduler
  hoists ready triggers on other queues, and a hoisted 1MB transfer
  measurably starves the critical chain).
- Supertile-0 split-m start: m0-3 run non-DR k0/k1 passes (first matmul
  needs only the 128KB k0 head chunk; the ~3.4us cold 1.2GHz HAM window is
  spent on these) then a DR(k2,k3) pass; m4-7 run pure DoubleRow pairs.
- Tail: final bf16 tiles narrow to 256 cols; their flushes go as 2-row
  pairs on the idle gpsimd queue (a trigger on scalar would block later
  PSUM copies on that sequencer); sync carries only the final 64KB flush.
"""

import os

import numpy as np
import ml_dtypes

import concourse.bass as bass
import concourse.mybir as mybir
from concourse import bacc
from concourse.tile import TileContext
from concourse.bass import ts
from concourse.bass_utils import run_bass_kernel_spmd

# Problem constants (hardcoded per spec)
B, D, C = 1024, 512, 100000
NCORES = 8
CS = C // NCORES          # 12500 classes per core
S, MARGIN, EPS = 30.0, 0.5, 1e-7

P = 128                   # partitions
KS = D // P               # 4 k-subtiles
MS = B // P               # 8 m-subtiles
NT = 512                  # n tile (one PSUM bank of fp32)
# device computes the largest NT-aligned prefix of each core's CS columns;
# the ragged remainder (212 cols/core) is computed on the host in fp32
DEV_CS = (CS // NT) * NT  # 12288
REM = CS - DEV_CS         # 212

# fp8 (DoubleRow) region
F8 = 3072                 # fp8 columns per core (6 x 512)
W8R = F8 - NT             # 2560 DoubleRow weight cols outside the head chunk
W8_ST1 = 2048             # cols in the 4-bank DoubleRow steady supertile
CBF = DEV_CS - F8         # 9216 bf16 columns
A_SCALE = 16.0            # emb8 = e4m3(A_SCALE * en)
B_SCALE = 16.0            # w8 = e4m3(B_SCALE * wn)
SC8 = S / (A_SCALE * B_SCALE)
HW8 = NT + B              # packed head8 row: [w8_k(512) | emb8_k(1024)]

BF16 = mybir.dt.bfloat16
F8E4 = mybir.dt.float8e4
DR = mybir.MatmulPerfMode.DoubleRow
_bf16_np = ml_dtypes.bfloat16
_f8_np = ml_dtypes.float8_e4m3  # TRN fp8e4 semantics (inf at S.1111.000)

# bf16 supertiles: 8 x 1024, one 512, then a 256+256 tail (small final
# tile keeps the exit-critical output drain short)
SUPERS_BF = (
    [(F8 + i * 2 * NT, 2 * NT) for i in range(8)]
    + [(F8 + 8192, NT), (F8 + 8704, NT // 2), (F8 + 8960, NT // 2)]
)


def build_nc():
    nc = bacc.Bacc(None, target_bir_lowering=False)
    head8 = nc.dram_tensor("head8", [D, HW8], F8E4, kind="ExternalInput")
    w8r = nc.dram_tensor("w8r", [D, W8R], F8E4, kind="ExternalInput")
    embT = nc.dram_tensor("embT", [D, B], BF16, kind="ExternalInput")
    wT = nc.dram_tensor("wT", [D, CBF], BF16, kind="ExternalInput")
    out = nc.dram_tensor("out", [B, DEV_CS], BF16, kind="ExternalOutput")

    with TileContext(nc) as tc:
        with (
            tc.tile_pool(name="e8", bufs=1) as e8pool,
            tc.tile_pool(name="w8", bufs=1) as w8pool,
            tc.tile_pool(name="eb", bufs=1) as ebpool,
            tc.tile_pool(name="w", bufs=4) as wpool,
            tc.tile_pool(name="o8", bufs=1) as o8pool,
            tc.tile_pool(name="o", bufs=5) as opool,
            tc.tile_pool(name="ps", bufs=8, space="PSUM") as pspool,
        ):
            head8_r = head8[:].rearrange("(ko p) x -> p ko x", p=P)
            w8r_r = w8r[:].rearrange("(ko p) c -> p ko c", p=P)
            embT_r = embT[:].rearrange("(ko p) b -> p ko b", p=P)
            wT_r = wT[:].rearrange("(ko p) c -> p ko c", p=P)
            out_r = out[:].rearrange("(mo p) c -> p mo c", p=P)

            head8sb = e8pool.tile([P, KS, HW8], F8E4, tag="h8", name="head8sb")
            w8sb = w8pool.tile([P, KS, W8R], F8E4, tag="w8", name="w8sb")
            embsb = ebpool.tile([P, KS, B], BF16, tag="eb", name="embsb")

            # --- input DMA triggers (order = queue FIFO order) ---
            # ALL early inputs ride sync in exact consumption order: within
            # one queue transfers complete FIFO, so the st0 critical chain
            # monopolizes the (slowly ramping) early DMA bandwidth. embT is
            # the only input on scalar, and it's queued behind the first st0
            # copies so its data can't dilute the early window.
            # st0 feed, in exact consumption order. Single-ko chunks keep
            # 1KB-run descriptors (pair transfers with sub-1KB runs measured
            # far slower); the first chunk is the minimal 128KB the first
            # non-DR matmul needs (w k0 + emb k0 m0-3).
            H0 = NT + P            # 640: w_k0 + emb k0 m0 — all matmul #1 needs
            H1 = NT + MS // 2 * P  # 1024: w_k + emb m0-3
            nc.sync.dma_start(out=head8sb[:, 0, 0:H0], in_=head8_r[:, 0, 0:H0])
            nc.sync.dma_start(out=head8sb[:, 0, H0:H1], in_=head8_r[:, 0, H0:H1])
            nc.sync.dma_start(out=head8sb[:, 1, 0:H1], in_=head8_r[:, 1, 0:H1])
            nc.sync.dma_start(out=head8sb[:, 2, 0:H1], in_=head8_r[:, 2, 0:H1])
            nc.sync.dma_start(out=head8sb[:, 3, 0:H1], in_=head8_r[:, 3, 0:H1])
            nc.sync.dma_start(
                out=head8sb[:, 0:2, H1:HW8], in_=head8_r[:, 0:2, H1:HW8]
            )
            nc.sync.dma_start(
                out=head8sb[:, 2:4, H1:HW8], in_=head8_r[:, 2:4, H1:HW8]
            )
            nc.sync.dma_start(out=w8sb[:, 0:2, :], in_=w8r_r[:, 0:2, :])
            nc.sync.dma_start(out=w8sb[:, 2:4, :], in_=w8r_r[:, 2:4, :])
            # bf16 embeddings ride sync BEHIND the fp8 critical chain: the
            # Tile scheduler hoists ready triggers, so a different queue
            # would start this 1MB transfer immediately and starve the
            # early window (measured: +2.3us on the first k0 pass)
            nc.sync.dma_start(out=embsb[:, :, :], in_=embT_r[:, :, :])

            def e8pair(k, m):  # [128, 2, 128] stationary pair slice
                return head8sb[:, k : k + 2, NT + m * P : NT + (m + 1) * P]

            def e8one(k, m):
                return head8sb[:, k, NT + m * P : NT + (m + 1) * P]

            def emb(k, m):  # bf16 stationary
                return embsb[:, k, m * P : (m + 1) * P]

            # ---- fp8 supertile 0 (cols 0:512): split-m start. m0-3 run
            # non-DR k0/k1 passes first — the first matmul needs only the
            # 128KB head chunk and the ~3.4us cold HAM window is spent on
            # these anyway — then everything still cold-free runs DoubleRow.
            ps0 = [
                pspool.tile([P, NT], mybir.dt.float32, tag="ps", name=f"ps8_0_{m}")
                for m in range(MS)
            ]
            MH = MS // 2
            for k in (0, 1):
                for m in range(MH):
                    nc.tensor.matmul(
                        ps0[m][:, :],
                        lhsT=e8one(k, m),
                        rhs=head8sb[:, k, 0:NT],
                        start=(k == 0),
                        stop=False,
                    )
            for m in range(MH):
                nc.tensor.matmul(
                    ps0[m][:, :],
                    lhsT=e8pair(2, m),
                    rhs=head8sb[:, 2:4, 0:NT],
                    start=False,
                    stop=True,
                    perf_mode=DR,
                )
            for j in (0, 1):
                for m in range(MH, MS):
                    nc.tensor.matmul(
                        ps0[m][:, :],
                        lhsT=e8pair(2 * j, m),
                        rhs=head8sb[:, 2 * j : 2 * j + 2, 0:NT],
                        start=(j == 0),
                        stop=(j == 1),
                        perf_mode=DR,
                    )
            o0 = o8pool.tile([P, MS, NT], BF16, tag="o8a", name="o0")
            for m in range(MS):
                if m % 2 == 0:
                    nc.scalar.mul(out=o0[:, m, :], in_=ps0[m][:, :], mul=SC8)
                else:
                    nc.vector.tensor_scalar_mul(o0[:, m, :], ps0[m][:, :], SC8)
                if m == MS // 2 - 1:
                    nc.scalar.dma_start(
                        out=out_r[:, 0 : MS // 2, 0:NT], in_=o0[:, 0 : MS // 2, :]
                    )
                elif m == MS - 1:
                    nc.scalar.dma_start(
                        out=out_r[:, MS // 2 : MS, 0:NT], in_=o0[:, MS // 2 : MS, :]
                    )

            # ---- fp8 supertile 1 (cols 512:2560): pure DoubleRow ----
            NT1 = W8_ST1 // NT  # 4 n-tiles per m
            o1 = o8pool.tile([P, MS, W8_ST1], BF16, tag="o8b", name="o1")
            for m in range(MS):
                pst = [
                    pspool.tile(
                        [P, NT], mybir.dt.float32, tag="ps", name=f"ps8_1_{m}_{n}"
                    )
                    for n in range(NT1)
                ]
                for j in (0, 1):
                    for n in range(NT1):
                        nc.tensor.matmul(
                            pst[n][:, :],
                            lhsT=e8pair(2 * j, m),
                            rhs=w8sb[:, 2 * j : 2 * j + 2, n * NT : (n + 1) * NT],
                            start=(j == 0),
                            stop=(j == 1),
                            perf_mode=DR,
                        )
                for n in range(NT1):
                    if (m + n) % 2 == 0:
                        nc.scalar.mul(
                            out=o1[:, m, n * NT : (n + 1) * NT],
                            in_=pst[n][:, :],
                            mul=SC8,
                        )
                    else:
                        nc.vector.tensor_scalar_mul(
                            o1[:, m, n * NT : (n + 1) * NT], pst[n][:, :], SC8
                        )
                if m == MS // 2 - 1:
                    nc.scalar.dma_start(
                        out=out_r[:, 0 : MS // 2, NT : NT + W8_ST1],
                        in_=o1[:, 0 : MS // 2, :],
                    )
                elif m == MS - 1:
                    nc.scalar.dma_start(
                        out=out_r[:, MS // 2 : MS, NT : NT + W8_ST1],
                        in_=o1[:, MS // 2 : MS, :],
                    )

            # ---- fp8 supertile 2 (cols 2560:3072): DoubleRow, 1 bank/m ----
            ST2 = NT + W8_ST1  # 2560, start col of st2
            o2 = o8pool.tile([P, MS, NT], BF16, tag="o8c", name="o2")
            for m in range(MS):
                ps2 = pspool.tile(
                    [P, NT], mybir.dt.float32, tag="ps", name=f"ps8_2_{m}"
                )
                for j in (0, 1):
                    nc.tensor.matmul(
                        ps2[:, :],
                        lhsT=e8pair(2 * j, m),
                        rhs=w8sb[:, 2 * j : 2 * j + 2, W8_ST1 : W8R],
                        start=(j == 0),
                        stop=(j == 1),
                        perf_mode=DR,
                    )
                if m % 2 == 0:
                    nc.scalar.mul(out=o2[:, m, :], in_=ps2[:, :], mul=SC8)
                else:
                    nc.vector.tensor_scalar_mul(o2[:, m, :], ps2[:, :], SC8)
                if m == MS // 2 - 1:
                    nc.scalar.dma_start(
                        out=out_r[:, 0 : MS // 2, ST2:F8],
                        in_=o2[:, 0 : MS // 2, :],
                    )
                elif m == MS - 1:
                    nc.scalar.dma_start(
                        out=out_r[:, MS // 2 : MS, ST2:F8],
                        in_=o2[:, MS // 2 : MS, :],
                    )

            # ---- bf16 supertiles: proven m-outer/k-inner steady state ----
            for idx, (n0, nw) in enumerate(SUPERS_BF):
                last_tile = idx == len(SUPERS_BF) - 1
                w_sb = wpool.tile([P, KS, 2 * NT], BF16, tag="w", name=f"w_{n0}")
                nc.sync.dma_start(
                    out=w_sb[:, :, :nw], in_=wT_r[:, :, n0 - F8 : n0 - F8 + nw]
                )
                o_sb = opool.tile([P, MS, 2 * NT], BF16, tag="o")
                for h in range(2):
                    h0 = h * NT
                    hw = min(NT, nw - h0)
                    if hw <= 0:
                        continue
                    for m in range(MS):
                        last_h = (h == 1) or (nw <= NT)
                        final_m = last_tile and m == MS - 1
                        if final_m:
                            # final row-block: two PSUM banks so its two
                            # PSUM->SBUF copies run in parallel on ACT and DVE
                            hh = hw // 2
                            psA = pspool.tile(
                                [P, NT], mybir.dt.float32, tag="ps",
                                name=f"ps_{n0}_{h}_{m}a",
                            )
                            psB = pspool.tile(
                                [P, NT], mybir.dt.float32, tag="ps",
                                name=f"ps_{n0}_{h}_{m}b",
                            )
                            for k in range(KS):
                                nc.tensor.matmul(
                                    psA[:, :hh],
                                    lhsT=emb(k, m),
                                    rhs=w_sb[:, k, h0 : h0 + hh],
                                    start=(k == 0),
                                    stop=(k == KS - 1),
                                )
                            for k in range(KS):
                                nc.tensor.matmul(
                                    psB[:, : hw - hh],
                                    lhsT=emb(k, m),
                                    rhs=w_sb[:, k, h0 + hh : h0 + hw],
                                    start=(k == 0),
                                    stop=(k == KS - 1),
                                )
                            nc.scalar.copy(
                                out=o_sb[:, m, h0 : h0 + hh], in_=psA[:, :hh]
                            )
                            nc.vector.tensor_copy(
                                out=o_sb[:, m, h0 + hh : h0 + hw],
                                in_=psB[:, : hw - hh],
                            )
                            nc.sync.dma_start(
                                out=out_r[:, m : m + 1, n0 : n0 + nw],
                                in_=o_sb[:, m : m + 1, :nw],
                            )
                            continue
                        ps = pspool.tile(
                            [P, NT], mybir.dt.float32, tag="ps",
                            name=f"ps_{n0}_{h}_{m}",
                        )
                        for k in range(KS):
                            nc.tensor.matmul(
                                ps[:, :hw],
                                lhsT=emb(k, m),
                                rhs=w_sb[:, k, h0 : h0 + hw],
                                start=(k == 0),
                                stop=(k == KS - 1),
                            )
                        if m % 2 == 0:
                            nc.scalar.copy(
                                out=o_sb[:, m, h0 : h0 + hw], in_=ps[:, :hw]
                            )
                        else:
                            nc.vector.tensor_copy(
                                out=o_sb[:, m, h0 : h0 + hw], in_=ps[:, :hw]
                            )
                        second_last = idx == len(SUPERS_BF) - 2
                        if last_h and last_tile:
                            # tail: flush in 2-row pairs on gpsimd (idle
                            # queue) — a trigger on scalar would block the
                            # later PSUM copies on that sequencer for 0.6us
                            # each; sync is reserved for the final m7 flush
                            if m % 2 == 1:
                                nc.gpsimd.dma_start(
                                    out=out_r[:, m - 1 : m + 1, n0 : n0 + nw],
                                    in_=o_sb[:, m - 1 : m + 1, :nw],
                                )
                            elif m == MS - 2:
                                nc.gpsimd.dma_start(
                                    out=out_r[:, m : m + 1, n0 : n0 + nw],
                                    in_=o_sb[:, m : m + 1, :nw],
                                )
                        elif not last_tile and last_h and m == MS // 2 - 1:
                            nc.scalar.dma_start(
                                out=out_r[:, 0 : MS // 2, n0 : n0 + nw],
                                in_=o_sb[:, 0 : MS // 2, :nw],
                            )
                        elif not last_tile and last_h and m == MS - 1:
                            eng = nc.gpsimd if second_last else nc.scalar
                            eng.dma_start(
                                out=out_r[:, MS // 2 : MS, n0 : n0 + nw],
                                in_=o_sb[:, MS // 2 : MS, :nw],
                            )
    nc.finalize()
    return nc


_NC_CACHE = []


def _get_nc():
    if not _NC_CACHE:
        _NC_CACHE.append(build_nc())
    return _NC_CACHE[0]


def _prep_in_maps(embeddings, weight):
    en = embeddings / np.maximum(
        np.linalg.norm(embeddings, axis=1, keepdims=True), 1e-12
    )
    wn = weight / np.maximum(np.linalg.norm(weight, axis=1, keepdims=True), 1e-12)
    embT_b = np.ascontiguousarray((S * en).T).astype(_bf16_np)  # [D, B]
    e8h = np.ascontiguousarray((A_SCALE * en).T).astype(_f8_np)  # [D, B]
    wTn = wn.T  # [D, C] view
    in_maps = []
    for i in range(NCORES):
        sh = wTn[:, i * CS : i * CS + DEV_CS]  # [D, DEV_CS]
        head8 = np.empty((D, HW8), dtype=_f8_np)
        head8[:, :NT] = (B_SCALE * sh[:, :NT]).astype(_f8_np)
        head8[:, NT:] = e8h
        w8r = np.ascontiguousarray(B_SCALE * sh[:, NT:F8]).astype(_f8_np)
        wT = np.ascontiguousarray(sh[:, F8:]).astype(_bf16_np)
        in_maps.append(
            {"head8": head8, "w8r": w8r, "embT": embT_b, "wT": wT}
        )
    return in_maps, en, wn


def run_device(embeddings, weight, **spmd_kwargs):
    """Runs the device part; returns (full S*cosine [B, C] fp32, raw results)."""
    if not spmd_kwargs.get("trace"):
        os.environ.setdefault("BASS_NEVER_TRACE", "1")
    nc = _get_nc()
    in_maps, en, wn = _prep_in_maps(embeddings, weight)
    try:
        res = run_bass_kernel_spmd(
            nc, in_maps, core_ids=list(range(NCORES)), **spmd_kwargs
        )
    except Exception:
        # rare transient NRT faults observed on this fleet; retry, and if
        # that fails too (e.g. profile hook wedged by the fault), retry
        # once more without tracing so correctness still returns
        try:
            res = run_bass_kernel_spmd(
                nc, in_maps, core_ids=list(range(NCORES)), **spmd_kwargs
            )
        except Exception:
            res = run_bass_kernel_spmd(
                nc, in_maps, core_ids=list(range(NCORES))
            )
    # ragged remainder columns (212 per core) in fp32 on the host
    rem_w = np.concatenate(
        [wn[i * CS + DEV_CS : (i + 1) * CS] for i in range(NCORES)], axis=0
    )  # [NCORES*REM, D]
    rem_out = (S * en) @ rem_w.T  # [B, NCORES*REM]
    out = np.empty((B, C), dtype=np.float32)
    for i in range(NCORES):
        out[:, i * CS : i * CS + DEV_CS] = np.asarray(
            res.results[i]["out"]
        ).astype(np.float32)
        out[:, i * CS + DEV_CS : (i + 1) * CS] = rem_out[
            :, i * REM : (i + 1) * REM
        ]
    return out, res, en, wn


def apply_margin(out, labels, en=None, wn=None):
    rows = np.arange(B)
    lab = np.asarray(labels).astype(np.int64)
    if en is not None and wn is not None:
        # exact fp32 label logits: fp8/bf16 noise never feeds the margin
        out[rows, lab] = S * np.einsum("bd,bd->b", en, wn[lab])
    c = np.clip(out[rows, lab] / S, -1.0 + EPS, 1.0 - EPS)
    out[rows, lab] = S * (c * np.cos(MARGIN) - np.sqrt(1.0 - c * c) * np.sin(MARGIN))
    return out


def kernel(embeddings, weight, labels):
    embeddings = np.asarray(embeddings, dtype=np.float32)
    weight = np.asarray(weight, dtype=np.float32)
    out, _, en, wn = run_device(embeddings, weight)
    return apply_margin(out, labels, en, wn)


# revision 24
# speedup vs baseline: 1.0051x; 1.0051x over previous
"""ArcFace head (B=1024, D=512, C=100000) on 8 TRN2 NeuronCores.

Sharding: tensor-parallel along num_classes (partial-FC ArcFace). Each core
computes a [B, 12288] slice of S*cosine; the ragged 212-col remainder and the
per-row margin are handled on the host.

Hybrid precision (key speed lever, tuned against the 2e-2 rel-err gate):
- cols 0:3072   : fp8 e4m3 with perf_mode=DoubleRow (2 k-subtiles per MM;
                  a DR matmul measures the same 216ns as a bf16 N=512 MM =
                  clean 2x throughput). Host pre-scales both operands by 16
                  (e4m3 normal range); the S/256 descale is folded into the
                  PSUM->SBUF copy (ACT mul / DVE tensor_scalar_mul).
                  Device rel-err matches the host ml_dtypes sim to 6
                  digits: 1.939e-2 (gate 2e-2, bf16-only is 2.9e-3).
- cols 3072:12288: bf16, the traced-gapless baseline structure (m-outer
                  k-inner supertiles of 1024, 216ns/MM warm roofline).
- label-column logits are recomputed exactly on host before the margin, so
  fp8 noise never feeds the acos/cos margin transform.

Edge schedule (from per-run NTFF traces: ~7.2us fixed NEFF init, each
dma_start trigger ~0.65us serial on its engine SEQUENCER, doorbell->data
~0.8us, early DMA bandwidth ramps slowly and sub-1KB-run descriptors load
far slower than 1KB+ runs):
- ALL early inputs ride the sync queue in exact consumption order (FIFO
  within one queue is the only ordering guarantee — the Tile scheduler
  hoists ready triggers on other queues, and a hoisted 1MB transfer
  measurably starves the critical chain).
- Supertile-0 split-m start: m0-3 run non-DR k0/k1 passes (first matmul
  needs only the 128KB k0 head chunk; the ~3.4us cold 1.2GHz HAM window is
  spent on these) then a DR(k2,k3) pass; m4-7 run pure DoubleRow pairs.
- Tail: final bf16 tiles narrow to 256 cols; their flushes go as 2-row
  pairs on the idle gpsimd queue (a trigger on scalar would block later
  PSUM copies on that sequencer); sync carries only the final 64KB flush.
"""

import os

import numpy as np
import ml_dtypes

import concourse.bass as bass
import concourse.mybir as mybir
from concourse import bacc
from concourse.tile import TileContext
from concourse.bass import ts
from concourse.bass_utils import run_bass_kernel_spmd

# Problem constants (hardcoded per spec)
B, D, C = 1024, 512, 100000
NCORES = 8
CS = C // NCORES          # 12500 classes per core
S, MARGIN, EPS = 30.0, 0.5, 1e-7

P = 128                   # partitions
KS = D // P               # 4 k-subtiles
MS = B // P               # 8 m-subtiles
NT = 512                  # n tile (one PSUM bank of fp32)
# device computes the largest NT-aligned prefix of each core's CS columns;
# the ragged remainder (212 cols/core) is computed on the host in fp32
DEV_CS = (CS // NT) * NT  # 12288
REM = CS - DEV_CS         # 212

# fp8 (DoubleRow) region
F8 = 3072                 # fp8 columns per core (6 x 512)
W8R = F8 - NT             # 2560 DoubleRow weight cols outside the head chunk
W8_ST1 = 2048             # cols in the 4-bank DoubleRow steady supertile
CBF = DEV_CS - F8         # 9216 bf16 columns
A_SCALE = 16.0            # emb8 = e4m3(A_SCALE * en)
B_SCALE = 16.0            # w8 = e4m3(B_SCALE * wn)
SC8 = S / (A_SCALE * B_SCALE)
HW8 = NT + B              # packed head8 row: [w8_k(512) | emb8_k(1024)]

BF16 = mybir.dt.bfloat16
F8E4 = mybir.dt.float8e4
DR = mybir.MatmulPerfMode.DoubleRow
_bf16_np = ml_dtypes.bfloat16
_f8_np = ml_dtypes.float8_e4m3  # TRN fp8e4 semantics (inf at S.1111.000)

# bf16 supertiles: 8 x 1024, one 512, then a 256+256 tail (small final
# tile keeps the exit-critical output drain short)
SUPERS_BF = (
    [(F8 + i * 2 * NT, 2 * NT) for i in range(8)]
    + [(F8 + 8192, NT), (F8 + 8704, NT // 2), (F8 + 8960, NT // 2)]
)


def build_nc():
    nc = bacc.Bacc(None, target_bir_lowering=False)
    head8 = nc.dram_tensor("head8", [D, HW8], F8E4, kind="ExternalInput")
    w8r = nc.dram_tensor("w8r", [D, W8R], F8E4, kind="ExternalInput")
    embT = nc.dram_tensor("embT", [D, B], BF16, kind="ExternalInput")
    wT = nc.dram_tensor("wT", [D, CBF], BF16, kind="ExternalInput")
    out = nc.dram_tensor("out", [B, DEV_CS], BF16, kind="ExternalOutput")

    with TileContext(nc) as tc:
        with (
            tc.tile_pool(name="e8", bufs=1) as e8pool,
            tc.tile_pool(name="w8", bufs=1) as w8pool,
            tc.tile_pool(name="eb", bufs=1) as ebpool,
            tc.tile_pool(name="w", bufs=4) as wpool,
            tc.tile_pool(name="o8", bufs=1) as o8pool,
            tc.tile_pool(name="o", bufs=5) as opool,
            tc.tile_pool(name="ps", bufs=8, space="PSUM") as pspool,
        ):
            head8_r = head8[:].rearrange("(ko p) x -> p ko x", p=P)
            w8r_r = w8r[:].rearrange("(ko p) c -> p ko c", p=P)
            embT_r = embT[:].rearrange("(ko p) b -> p ko b", p=P)
            wT_r = wT[:].rearrange("(ko p) c -> p ko c", p=P)
            out_r = out[:].rearrange("(mo p) c -> p mo c", p=P)

            head8sb = e8pool.tile([P, KS, HW8], F8E4, tag="h8", name="head8sb")
            w8sb = w8pool.tile([P, KS, W8R], F8E4, tag="w8", name="w8sb")
            embsb = ebpool.tile([P, KS, B], BF16, tag="eb", name="embsb")

            # --- input DMA triggers (order = queue FIFO order) ---
            # ALL early inputs ride sync in exact consumption order: within
            # one queue transfers complete FIFO, so the st0 critical chain
            # monopolizes the (slowly ramping) early DMA bandwidth. embT is
            # the only input on scalar, and it's queued behind the first st0
            # copies so its data can't dilute the early window.
            # st0 feed, in exact consumption order. Single-ko chunks keep
            # 1KB-run descriptors (pair transfers with sub-1KB runs measured
            # far slower); the first chunk is the minimal 128KB the first
            # non-DR matmul needs (w k0 + emb k0 m0-3).
            H1 = NT + MS // 2 * P  # 1024: w_k + emb m0-3
            nc.sync.dma_start(out=head8sb[:, 0, 0:H1], in_=head8_r[:, 0, 0:H1])
            nc.sync.dma_start(out=head8sb[:, 1, 0:H1], in_=head8_r[:, 1, 0:H1])
            nc.sync.dma_start(out=head8sb[:, 2, 0:H1], in_=head8_r[:, 2, 0:H1])
            nc.sync.dma_start(out=head8sb[:, 3, 0:H1], in_=head8_r[:, 3, 0:H1])
            nc.sync.dma_start(
                out=head8sb[:, 0:2, H1:HW8], in_=head8_r[:, 0:2, H1:HW8]
            )
            nc.sync.dma_start(
                out=head8sb[:, 2:4, H1:HW8], in_=head8_r[:, 2:4, H1:HW8]
            )
            nc.sync.dma_start(out=w8sb[:, 0:2, :], in_=w8r_r[:, 0:2, :])
            nc.sync.dma_start(out=w8sb[:, 2:4, :], in_=w8r_r[:, 2:4, :])
            # bf16 embeddings ride sync BEHIND the fp8 critical chain: the
            # Tile scheduler hoists ready triggers, so a different queue
            # would start this 1MB transfer immediately and starve the
            # early window (measured: +2.3us on the first k0 pass)
            nc.sync.dma_start(out=embsb[:, :, :], in_=embT_r[:, :, :])

            def e8pair(k, m):  # [128, 2, 128] stationary pair slice
                return head8sb[:, k : k + 2, NT + m * P : NT + (m + 1) * P]

            def e8one(k, m):
                return head8sb[:, k, NT + m * P : NT + (m + 1) * P]

            def emb(k, m):  # bf16 stationary
                return embsb[:, k, m * P : (m + 1) * P]

            # ---- fp8 supertile 0 (cols 0:512): split-m start. m0-3 run
            # non-DR k0/k1 passes first — the first matmul needs only the
            # 128KB head chunk and the ~3.4us cold HAM window is spent on
            # these anyway — then everything still cold-free runs DoubleRow.
            ps0 = [
                pspool.tile([P, NT], mybir.dt.float32, tag="ps", name=f"ps8_0_{m}")
                for m in range(MS)
            ]
            MH = MS // 2
            for k in (0, 1):
                for m in range(MH):
                    nc.tensor.matmul(
                        ps0[m][:, :],
                        lhsT=e8one(k, m),
                        rhs=head8sb[:, k, 0:NT],
                        start=(k == 0),
                        stop=False,
                    )
            for m in range(MH):
                nc.tensor.matmul(
                    ps0[m][:, :],
                    lhsT=e8pair(2, m),
                    rhs=head8sb[:, 2:4, 0:NT],
                    start=False,
                    stop=True,
                    perf_mode=DR,
                )
            for j in (0, 1):
                for m in range(MH, MS):
                    nc.tensor.matmul(
                        ps0[m][:, :],
                        lhsT=e8pair(2 * j, m),
                        rhs=head8sb[:, 2 * j : 2 * j + 2, 0:NT],
                        start=(j == 0),
                        stop=(j == 1),
                        perf_mode=DR,
                    )
            o0 = o8pool.tile([P, MS, NT], BF16, tag="o8a", name="o0")
            for m in range(MS):
                if m % 2 == 0:
                    nc.scalar.mul(out=o0[:, m, :], in_=ps0[m][:, :], mul=SC8)
                else:
                    nc.vector.tensor_scalar_mul(o0[:, m, :], ps0[m][:, :], SC8)
                if m == MS // 2 - 1:
                    nc.scalar.dma_start(
                        out=out_r[:, 0 : MS // 2, 0:NT], in_=o0[:, 0 : MS // 2, :]
                    )
                elif m == MS - 1:
                    nc.scalar.dma_start(
                        out=out_r[:, MS // 2 : MS, 0:NT], in_=o0[:, MS // 2 : MS, :]
                    )

            # ---- fp8 supertile 1 (cols 512:2560): pure DoubleRow ----
            NT1 = W8_ST1 // NT  # 4 n-tiles per m
            o1 = o8pool.tile([P, MS, W8_ST1], BF16, tag="o8b", name="o1")
            for m in range(MS):
                pst = [
                    pspool.tile(
                        [P, NT], mybir.dt.float32, tag="ps", name=f"ps8_1_{m}_{n}"
                    )
                    for n in range(NT1)
                ]
                for j in (0, 1):
                    for n in range(NT1):
                        nc.tensor.matmul(
                            pst[n][:, :],
                            lhsT=e8pair(2 * j, m),
                            rhs=w8sb[:, 2 * j : 2 * j + 2, n * NT : (n + 1) * NT],
                            start=(j == 0),
                            stop=(j == 1),
                            perf_mode=DR,
                        )
                for n in range(NT1):
                    if (m + n) % 2 == 0:
                        nc.scalar.mul(
                            out=o1[:, m, n * NT : (n + 1) * NT],
                            in_=pst[n][:, :],
                            mul=SC8,
                        )
                    else:
                        nc.vector.tensor_scalar_mul(
                            o1[:, m, n * NT : (n + 1) * NT], pst[n][:, :], SC8
                        )
                if m == MS // 2 - 1:
                    nc.scalar.dma_start(
                        out=out_r[:, 0 : MS // 2, NT : NT + W8_ST1],
                        in_=o1[:, 0 : MS // 2, :],
                    )
                elif m == MS - 1:
                    nc.scalar.dma_start(
                        out=out_r[:, MS // 2 : MS, NT : NT + W8_ST1],
                        in_=o1[:, MS // 2 : MS, :],
                    )

            # ---- fp8 supertile 2 (cols 2560:3072): DoubleRow, 1 bank/m ----
            ST2 = NT + W8_ST1  # 2560, start col of st2
            o2 = o8pool.tile([P, MS, NT], BF16, tag="o8c", name="o2")
            for m in range(MS):
                ps2 = pspool.tile(
                    [P, NT], mybir.dt.float32, tag="ps", name=f"ps8_2_{m}"
                )
                for j in (0, 1):
                    nc.tensor.matmul(
                        ps2[:, :],
                        lhsT=e8pair(2 * j, m),
                        rhs=w8sb[:, 2 * j : 2 * j + 2, W8_ST1 : W8R],
                        start=(j == 0),
                        stop=(j == 1),
                        perf_mode=DR,
                    )
                if m % 2 == 0:
                    nc.scalar.mul(out=o2[:, m, :], in_=ps2[:, :], mul=SC8)
                else:
                    nc.vector.tensor_scalar_mul(o2[:, m, :], ps2[:, :], SC8)
                if m == MS // 2 - 1:
                    nc.scalar.dma_start(
                        out=out_r[:, 0 : MS // 2, ST2:F8],
                        in_=o2[:, 0 : MS // 2, :],
                    )
                elif m == MS - 1:
                    nc.scalar.dma_start(
                        out=out_r[:, MS // 2 : MS, ST2:F8],
                        in_=o2[:, MS // 2 : MS, :],
                    )

            # ---- bf16 supertiles: proven m-outer/k-inner steady state ----
            for idx, (n0, nw) in enumerate(SUPERS_BF):
                last_tile = idx == len(SUPERS_BF) - 1
                w_sb = wpool.tile([P, KS, 2 * NT], BF16, tag="w", name=f"w_{n0}")
                nc.sync.dma_start(
                    out=w_sb[:, :, :nw], in_=wT_r[:, :, n0 - F8 : n0 - F8 + nw]
                )
                o_sb = opool.tile([P, MS, 2 * NT], BF16, tag="o")
                for h in range(2):
                    h0 = h * NT
                    hw = min(NT, nw - h0)
                    if hw <= 0:
                        continue
                    for m in range(MS):
                        last_h = (h == 1) or (nw <= NT)
                        final_m = last_tile and m == MS - 1
                        if final_m:
                            # final row-block: two PSUM banks so its two
                            # PSUM->SBUF copies run in parallel on ACT and DVE
                            hh = hw // 2
                            psA = pspool.tile(
                                [P, NT], mybir.dt.float32, tag="ps",
                                name=f"ps_{n0}_{h}_{m}a",
                            )
                            psB = pspool.tile(
                                [P, NT], mybir.dt.float32, tag="ps",
                                name=f"ps_{n0}_{h}_{m}b",
                            )
                            for k in range(KS):
                                nc.tensor.matmul(
                                    psA[:, :hh],
                                    lhsT=emb(k, m),
                                    rhs=w_sb[:, k, h0 : h0 + hh],
                                    start=(k == 0),
                                    stop=(k == KS - 1),
                                )
                            for k in range(KS):
                                nc.tensor.matmul(
                                    psB[:, : hw - hh],
                                    lhsT=emb(k, m),
                                    rhs=w_sb[:, k, h0 + hh : h0 + hw],
                                    start=(k == 0),
                                    stop=(k == KS - 1),
                                )
                            nc.scalar.copy(
                                out=o_sb[:, m, h0 : h0 + hh], in_=psA[:, :hh]
                            )
                            nc.vector.tensor_copy(
                                out=o_sb[:, m, h0 + hh : h0 + hw],
                                in_=psB[:, : hw - hh],
                            )
                            nc.sync.dma_start(
                                out=out_r[:, m : m + 1, n0 : n0 + nw],
                                in_=o_sb[:, m : m + 1, :nw],
                            )
                            continue
                        ps = pspool.tile(
                            [P, NT], mybir.dt.float32, tag="ps",
                            name=f"ps_{n0}_{h}_{m}",
                        )
                        for k in range(KS):
                            nc.tensor.matmul(
                                ps[:, :hw],
                                lhsT=emb(k, m),
                                rhs=w_sb[:, k, h0 : h0 + hw],
                                start=(k == 0),
                                stop=(k == KS - 1),
                            )
                        if m % 2 == 0:
                            nc.scalar.copy(
                                out=o_sb[:, m, h0 : h0 + hw], in_=ps[:, :hw]
                            )
                        else:
                            nc.vector.tensor_copy(
                                out=o_sb[:, m, h0 : h0 + hw], in_=ps[:, :hw]
                            )
                        second_last = idx == len(SUPERS_BF) - 2
                        if last_h and last_tile:
                            # tail: flush in 2-row pairs on gpsimd (idle
                            # queue) — a trigger on scalar would block the
                            # later PSUM copies on that sequencer for 0.6us
                            # each; sync is reserved for the final m7 flush
                            if m % 2 == 1:
                                # alternate scalar/gpsimd so the tail drain
                                # splits across two queues (one queue alone
                                # serialized ~700KB and gated the exit 4.7us
                                # past the last matmul)
                                eng = nc.scalar if m % 4 == 1 else nc.gpsimd
                                eng.dma_start(
                                    out=out_r[:, m - 1 : m + 1, n0 : n0 + nw],
                                    in_=o_sb[:, m - 1 : m + 1, :nw],
                                )
                            elif m == MS - 2:
                                nc.gpsimd.dma_start(
                                    out=out_r[:, m : m + 1, n0 : n0 + nw],
                                    in_=o_sb[:, m : m + 1, :nw],
                                )
                        elif not last_tile and last_h and m == MS // 2 - 1:
                            nc.scalar.dma_start(
                                out=out_r[:, 0 : MS // 2, n0 : n0 + nw],
                                in_=o_sb[:, 0 : MS // 2, :nw],
                            )
                        elif not last_tile and last_h and m == MS - 1:
                            eng = nc.gpsimd if second_last else nc.scalar
                            eng.dma_start(
                                out=out_r[:, MS // 2 : MS, n0 : n0 + nw],
                                in_=o_sb[:, MS // 2 : MS, :nw],
                            )
    nc.finalize()
    return nc


_NC_CACHE = []


def _get_nc():
    if not _NC_CACHE:
        _NC_CACHE.append(build_nc())
    return _NC_CACHE[0]


def _prep_in_maps(embeddings, weight):
    en = embeddings / np.maximum(
        np.linalg.norm(embeddings, axis=1, keepdims=True), 1e-12
    )
    wn = weight / np.maximum(np.linalg.norm(weight, axis=1, keepdims=True), 1e-12)
    embT_b = np.ascontiguousarray((S * en).T).astype(_bf16_np)  # [D, B]
    e8h = np.ascontiguousarray((A_SCALE * en).T).astype(_f8_np)  # [D, B]
    wTn = wn.T  # [D, C] view
    in_maps = []
    for i in range(NCORES):
        sh = wTn[:, i * CS : i * CS + DEV_CS]  # [D, DEV_CS]
        head8 = np.empty((D, HW8), dtype=_f8_np)
        head8[:, :NT] = (B_SCALE * sh[:, :NT]).astype(_f8_np)
        head8[:, NT:] = e8h
        w8r = np.ascontiguousarray(B_SCALE * sh[:, NT:F8]).astype(_f8_np)
        wT = np.ascontiguousarray(sh[:, F8:]).astype(_bf16_np)
        in_maps.append(
            {"head8": head8, "w8r": w8r, "embT": embT_b, "wT": wT}
        )
    return in_maps, en, wn


def run_device(embeddings, weight, **spmd_kwargs):
    """Runs the device part; returns (full S*cosine [B, C] fp32, raw results)."""
    if not spmd_kwargs.get("trace"):
        os.environ.setdefault("BASS_NEVER_TRACE", "1")
    nc = _get_nc()
    in_maps, en, wn = _prep_in_maps(embeddings, weight)
    try:
        res = run_bass_kernel_spmd(
            nc, in_maps, core_ids=list(range(NCORES)), **spmd_kwargs
        )
    except Exception:
        # rare transient NRT faults observed on this fleet; retry, and if
        # that fails too (e.g. profile hook wedged by the fault), retry
        # once more without tracing so correctness still returns
        try:
            res = run_bass_kernel_spmd(
                nc, in_maps, core_ids=list(range(NCORES)), **spmd_kwargs
            )
        except Exception:
            res = run_bass_kernel_spmd(
                nc, in_maps, core_ids=list(range(NCORES))
            )
    # ragged remainder columns (212 per core) in fp32 on the host
    rem_w = np.concatenate(
        [wn[i * CS + DEV_CS : (i + 1) * CS] for i in range(NCORES)], axis=0
    )  # [NCORES*REM, D]
    rem_out = (S * en) @ rem_w.T  # [B, NCORES*REM]
    out = np.empty((B, C), dtype=np.float32)
    for i in range(NCORES):
        out[:, i * CS : i * CS + DEV_CS] = np.asarray(
            res.results[i]["out"]
        ).astype(np.float32)
        out[:, i * CS + DEV_CS : (i + 1) * CS] = rem_out[
            :, i * REM : (i + 1) * REM
        ]
    return out, res, en, wn


def apply_margin(out, labels, en=None, wn=None):
    rows = np.arange(B)
    lab = np.asarray(labels).astype(np.int64)
    if en is not None and wn is not None:
        # exact fp32 label logits: fp8/bf16 noise never feeds the margin
        out[rows, lab] = S * np.einsum("bd,bd->b", en, wn[lab])
    c = np.clip(out[rows, lab] / S, -1.0 + EPS, 1.0 - EPS)
    out[rows, lab] = S * (c * np.cos(MARGIN) - np.sqrt(1.0 - c * c) * np.sin(MARGIN))
    return out


def kernel(embeddings, weight, labels):
    embeddings = np.asarray(embeddings, dtype=np.float32)
    weight = np.asarray(weight, dtype=np.float32)
    out, _, en, wn = run_device(embeddings, weight)
    return apply_margin(out, labels, en, wn)
